# revision 4
# baseline (speedup 1.0000x reference)
"""Trainium2 Bass kernel for nn_ConstraintLayer (batched equality-constrained QP layer).

Math: the reference solves  M @ sol_i = [2*y_i; b_i]  for every batch row i,
with the SAME KKT matrix M = [[2I, A^T], [A, 0]] (80x80).  Since M is fixed,
    y_star = [2y, b] @ (M^{-1}[:64, :])^T  =  y @ Gy + b @ Gb
with Gy = 2*Minv[:64,:64].T (64x64) and Gb = Minv[:64,64:].T (16x64),
i.e. one skinny (batch,80)@(80,64) matmul — memory bound.

Distribution: pure data parallelism; the batch (1048576 rows) is split into 8
shards of 131072 rows, one per NeuronCore.  The tiny Gy/Gb factors are
precomputed once on host (float64 inverse of the 80x80 block matrix) and
replicated to every core.

Precision: the correctness gate is rel-err < 2e-2; a single fp16 pass
(fp16 inputs/weights, fp32 PSUM accumulate, fp16 output) measures ~7e-4 —
30x inside the gate — and halves HBM traffic vs an fp32-accurate kernel:
36 MB/core total (16 MB Y + 4 MB B + 16 MB out) against the ~358 GB/s
per-core HBM roofline (~101 us).

Device layout (per core): the host pre-transposes each shard into
feature-major blocks so that every DMA is a contiguous [128-partition x 8KB]
1MB transfer, and TensorE consumes 512-column moving tiles directly.
Batch is processed in chunks of 512 rows, packed in PAIRS so each PSUM bank
[128, 512] holds two chunks (even chunk -> partitions 0-63, odd -> 64-127).
Per block of 8 pairs: 8 Y matmuls (K=128, stationary blockdiag(Gy)) then 8 B
matmuls (K=32, stationary Wb) accumulating into the same 8 PSUM banks — two
stationary-weight swaps per block instead of 16 — then one VectorE
PSUM->SBUF fp16 copy per pair and a contiguous 1MB fp16 DMA out; the host
inverts the packing.
"""

import numpy as np

BATCH = 1048576
IN_DIM = 64
OUT_DIM = 16
N_CORES = 8
SHARD = BATCH // N_CORES        # 131072
CHUNK = 512                     # batch rows per matmul (one PSUM bank col-span)
PAIRS_PER_YBLK = 16             # Y block [128, 8192] f16 = 16 pairs = 32 chunks
N_YBLK = SHARD // (2 * CHUNK * PAIRS_PER_YBLK)   # 8
N_BBLK = N_YBLK                 # B block [32, 8192] f16, same cadence as Y blocks
YCOLS = 512 * PAIRS_PER_YBLK    # 8192

_prog_cache = {}
last_results = None             # BassKernelResults of the most recent run (for test harness)


def _build_weights(A):
    """Host precompute of the stationary matrices (float64 inverse, fp16)."""
    m, n = A.shape  # (16, 64)
    A64 = np.asarray(A, dtype=np.float64)
    M = np.zeros((n + m, n + m))
    M[:n, :n] = 2.0 * np.eye(n)
    M[:n, n:] = A64.T
    M[n:, :n] = A64
    Minv = np.linalg.inv(M)
    Gy = (2.0 * Minv[:n, :n].T).astype(np.float16)   # (64, 64)
    Gb = (Minv[:n, n:].T).astype(np.float16)         # (16, 64)

    # Wy [128,128] = blockdiag(Gy): even chunk -> out partitions 0-63,
    # odd chunk -> 64-127, both in one K=128 matmul.
    Wy = np.zeros((128, 128), np.float16)
    Wy[:64, :64] = Gy
    Wy[64:, 64:] = Gy
    # Wb [32,128]: rows 0-15 b_even -> Gb @ cols 0:64, rows 16-31 b_odd -> cols 64:128.
    Wb = np.zeros((32, 128), np.float16)
    Wb[0:16, 0:64] = Gb
    Wb[16:32, 64:128] = Gb
    return Wy, Wb


def _pack_y(ys):
    # (131072, 64) f16 -> blocks (16, 128, 4096); partition = 64*parity + f,
    # col = 512*pairidx + s  (chunk c = 16*yb + 2*pairidx + parity)
    return np.ascontiguousarray(
        ys.reshape(N_YBLK, PAIRS_PER_YBLK, 2, CHUNK, 64).transpose(0, 2, 4, 1, 3)
    ).reshape(N_YBLK, 128, YCOLS)


def _pack_b(bh):
    # (131072, 16) f16 -> blocks (16, 32, 4096);
    # partition = 16*parity + i, col = 512*pairidx + s
    return np.ascontiguousarray(
        bh.reshape(N_BBLK, PAIRS_PER_YBLK, 2, CHUNK, 16).transpose(0, 2, 4, 1, 3)
    ).reshape(N_BBLK, 32, YCOLS)


def _unpack_out(ob):
    # inverse of _pack_y with feature dim 64: (16, 128, 4096) f16 -> (131072, 64)
    return np.ascontiguousarray(
        ob.reshape(N_YBLK, 2, 64, PAIRS_PER_YBLK, CHUNK).transpose(0, 3, 1, 4, 2)
    ).reshape(SHARD, 64)


def _build_program():
    import concourse.bacc as bacc
    import concourse.mybir as mybir
    import concourse.tile as tile

    f32 = mybir.dt.float32
    f16 = mybir.dt.float16
    nc = bacc.Bacc("TRN2")
    Yh_d = nc.dram_tensor("Yh", (N_YBLK, 128, YCOLS), f16, kind="ExternalInput")
    B_d = nc.dram_tensor("Bt", (N_BBLK, 32, YCOLS), f16, kind="ExternalInput")
    Wy_d = nc.dram_tensor("Wy", (128, 128), f16, kind="ExternalInput")
    Wb_d = nc.dram_tensor("Wb", (32, 128), f16, kind="ExternalInput")
    Ot = nc.dram_tensor("Ot", (N_YBLK, 128, YCOLS), f16, kind="ExternalOutput")

    with tile.TileContext(nc) as tc:
        with (
            tc.tile_pool(name="wpool", bufs=1) as wpool,
            tc.tile_pool(name="ypool", bufs=3) as ypool,
            tc.tile_pool(name="bpool", bufs=3) as bpool,
            tc.tile_pool(name="opool", bufs=3) as opool,
            tc.tile_pool(name="pspool", bufs=8, space="PSUM") as pspool,
        ):
            # Weights + B blocks go through the scalar-engine HWDGE ring, Y
            # blocks through the sync ring, outputs through SWDGE — three
            # independent FIFOs so a B-block load never queues behind 1MB of
            # Y traffic.
            wy = wpool.tile([128, 128], f16)
            nc.scalar.dma_start(wy[:], Wy_d[:])
            wb = wpool.tile([32, 128], f16)
            nc.scalar.dma_start(wb[:], Wb_d[:])

            for yb in range(N_YBLK):
                yh_t = ypool.tile([128, YCOLS], f16, tag="yh")
                nc.sync.dma_start(yh_t[:], Yh_d[yb])
                btile = bpool.tile([32, YCOLS], f16, tag="bt")
                nc.scalar.dma_start(btile[:], B_d[yb])
                otile = opool.tile([128, YCOLS], f16, tag="ot")
                for pi in range(PAIRS_PER_YBLK):
                    cols = slice(512 * pi, 512 * (pi + 1))
                    ps = pspool.tile([128, CHUNK], f32)
                    nc.tensor.matmul(ps[:], wy[:], yh_t[:, cols],
                                     start=True, stop=False)           # y @ Gy (both parities)
                    nc.tensor.matmul(ps[:], wb[:], btile[:, cols],
                                     start=False, stop=True)           # + b @ Gb (K=32)
                    # split PSUM->SBUF casts across the two free compute
                    # engines so neither becomes the dependency tail
                    if pi % 2 == 0:
                        nc.vector.tensor_copy(otile[:, cols], ps[:])
                    else:
                        nc.scalar.copy(otile[:, cols], ps[:])
                nc.gpsimd.dma_start(Ot[yb], otile[:])
    nc.compile()  # bacc passes: split sync waits to HW limits, alloc regs, DCE
    return nc


def _get_program():
    if "nc" not in _prog_cache:
        _prog_cache["nc"] = _build_program()
    return _prog_cache["nc"]


def kernel(y, A, b):
    global last_results
    from concourse.bass_utils import run_bass_kernel_spmd

    y = np.ascontiguousarray(np.asarray(y, dtype=np.float32))
    b = np.ascontiguousarray(np.asarray(b, dtype=np.float32))
    A = np.asarray(A, dtype=np.float32)
    assert y.shape == (BATCH, IN_DIM) and b.shape == (BATCH, OUT_DIM)

    Wy, Wb = _build_weights(A)
    yh = y.astype(np.float16)
    bh = b.astype(np.float16)

    in_maps = []
    for core in range(N_CORES):
        sl = slice(core * SHARD, (core + 1) * SHARD)
        in_maps.append({
            "Yh": _pack_y(yh[sl]),
            "Bt": _pack_b(bh[sl]),
            "Wy": Wy, "Wb": Wb,
        })

    nc = _get_program()
    res = run_bass_kernel_spmd(nc, in_maps, core_ids=list(range(N_CORES)))
    last_results = res

    out = np.empty((BATCH, IN_DIM), np.float32)
    for core in range(N_CORES):
        out[core * SHARD:(core + 1) * SHARD] = _unpack_out(res.results[core]["Ot"])
    return out


# revision 5
# speedup vs baseline: 1.3097x; 1.3097x over previous
"""Trainium2 Bass kernel for nn_ConstraintLayer (batched equality-constrained QP layer).

Math: the reference solves  M @ sol_i = [2*y_i; b_i]  for every batch row i,
with the SAME KKT matrix M = [[2I, A^T], [A, 0]] (80x80).  Since M is fixed,
    y_star = [2y, b] @ (M^{-1}[:64, :])^T  =  [y, b] @ Wc
with Wc = [Gy; Gb] (80x64), Gy = 2*Minv[:64,:64].T, Gb = Minv[:64,64:].T —
one skinny (batch,80)@(80,64) matmul, memory bound.

Distribution: pure data parallelism; the batch (1048576 rows) is split into 8
shards of 131072 rows, one per NeuronCore.  Wc is precomputed once on host
(float64 inverse) and replicated to every core.

Precision: the correctness gate is rel-err < 2e-2; a single fp16 pass
(fp16 inputs/weights, fp32 PSUM accumulate, fp16 output) measures ~7e-4 —
30x inside the gate — at 36 MB/core HBM traffic (20 MB in + 16 MB out)
against the ~358 GB/s per-core HBM roofline (~101 us).

Device layout (per core): the host packs each 512-row chunk feature-major as
an 80-partition moving tile ([64 y feats; 16 b feats] x 512 batch columns),
16 chunks per 2.5 MB block so every input DMA is a contiguous
[80-partition x 16KB] transfer and TensorE consumes the (80,512) tiles
directly — y and b arrive in ONE stream.

TensorE uses 128x64 COLUMN TILING (tile_position via PSUM base partition):
the K=80, M=64 stationary Wc is loaded into both column halves of the PE
array; even chunks stream through array cols 0-63 into PSUM partitions 0-63
while odd chunks stream through cols 64-127 into partitions 64-127
CONCURRENTLY — 2 moving columns/cycle, halving PE busy time vs a paired
K=128 blockdiag scheme, with no separate b matmul and a single 64-column
stationary reload per matmul.  Each PSUM bank [128,512] then holds two
finished chunks; PSUM->SBUF fp16 casts alternate between VectorE and
ScalarE (neither becomes the tail), and a contiguous 1MB fp16 DMA writes
the block out; the host inverts the packing.
"""

import numpy as np

BATCH = 1048576
IN_DIM = 64
OUT_DIM = 16
N_CORES = 8
SHARD = BATCH // N_CORES        # 131072
CHUNK = 512                     # batch rows per matmul (one PSUM half-bank col-span)
CH_PER_BLK = 16                 # chunks per input block: [80, 8192] f16 = 2.5 MB
N_BLK = SHARD // (CHUNK * CH_PER_BLK)   # 16
ICOLS = CHUNK * CH_PER_BLK      # 8192
OCOLS = ICOLS // 2              # 4096 (two chunks share a PSUM bank / out col-span)

_prog_cache = {}
last_results = None             # BassKernelResults of the most recent run (for test harness)


def _build_weights(A):
    """Host precompute of the stationary matrix (float64 inverse, fp16)."""
    m, n = A.shape  # (16, 64)
    A64 = np.asarray(A, dtype=np.float64)
    M = np.zeros((n + m, n + m))
    M[:n, :n] = 2.0 * np.eye(n)
    M[:n, n:] = A64.T
    M[n:, :n] = A64
    Minv = np.linalg.inv(M)
    Gy = (2.0 * Minv[:n, :n].T)          # (64, 64):  out = y @ Gy + b @ Gb
    Gb = (Minv[:n, n:].T)                # (16, 64)
    return np.concatenate([Gy, Gb], axis=0).astype(np.float16)   # Wc (80, 64)


def _pack_in(yh, bh):
    # (131072, 64)+(131072, 16) f16 -> blocks (16, 80, 8192);
    # partition = feature (0-63 y, 64-79 b), col = 512*chunk + s
    yv = yh.reshape(N_BLK, CH_PER_BLK, CHUNK, 64).transpose(0, 3, 1, 2)
    bv = bh.reshape(N_BLK, CH_PER_BLK, CHUNK, 16).transpose(0, 3, 1, 2)
    return np.ascontiguousarray(
        np.concatenate([yv.reshape(N_BLK, 64, ICOLS),
                        bv.reshape(N_BLK, 16, ICOLS)], axis=1))


def _unpack_out(ob):
    # (16, 128, 4096) f16 -> (131072, 64); partition = 64*(chunk%2) + feat,
    # col = 512*(chunk//2) + s  within each block of 16 chunks
    return np.ascontiguousarray(
        ob.reshape(N_BLK, 2, 64, CH_PER_BLK // 2, CHUNK).transpose(0, 3, 1, 4, 2)
    ).reshape(SHARD, 64)


def _build_program():
    import concourse.bacc as bacc
    import concourse.mybir as mybir
    import concourse.tile as tile

    f32 = mybir.dt.float32
    f16 = mybir.dt.float16
    nc = bacc.Bacc("TRN2")
    In_d = nc.dram_tensor("In", (N_BLK, 80, ICOLS), f16, kind="ExternalInput")
    Wc_d = nc.dram_tensor("Wc", (80, 64), f16, kind="ExternalInput")
    Ot = nc.dram_tensor("Ot", (N_BLK, 128, OCOLS), f16, kind="ExternalOutput")

    with tile.TileContext(nc) as tc:
        with (
            tc.tile_pool(name="wpool", bufs=1) as wpool,
            tc.tile_pool(name="ipool", bufs=3) as ipool,
            tc.tile_pool(name="opool", bufs=3) as opool,
            tc.tile_pool(name="pspool", bufs=8, space="PSUM") as pspool,
        ):
            wc = wpool.tile([80, 64], f16)
            nc.scalar.dma_start(wc[:], Wc_d[:])

            for blk in range(N_BLK):
                itile = ipool.tile([80, ICOLS], f16, tag="in")
                nc.sync.dma_start(itile[:], In_d[blk])
                otile = opool.tile([128, OCOLS], f16, tag="ot")
                for i in range(CH_PER_BLK // 2):
                    cols_e = slice((2 * i) * CHUNK, (2 * i + 1) * CHUNK)
                    cols_o = slice((2 * i + 1) * CHUNK, (2 * i + 2) * CHUNK)
                    ocols = slice(i * CHUNK, (i + 1) * CHUNK)
                    ps = pspool.tile([128, CHUNK], f32)
                    # 128x64 column tiling: same stationary in both column
                    # halves; the two chunk streams run CONCURRENTLY.
                    nc.tensor.matmul(ps[0:64, :], wc[:], itile[:, cols_e],
                                     start=True, stop=True)
                    nc.tensor.matmul(ps[64:128, :], wc[:], itile[:, cols_o],
                                     start=True, stop=True)
                    # split PSUM->SBUF casts across the two free compute
                    # engines so neither becomes the dependency tail
                    if i % 2 == 0:
                        nc.vector.tensor_copy(otile[:, ocols], ps[:])
                    else:
                        nc.scalar.copy(otile[:, ocols], ps[:])
                nc.gpsimd.dma_start(Ot[blk], otile[:])
    nc.compile()  # bacc passes: split sync waits to HW limits, alloc regs, DCE
    return nc


def _get_program():
    if "nc" not in _prog_cache:
        _prog_cache["nc"] = _build_program()
    return _prog_cache["nc"]


def kernel(y, A, b):
    global last_results
    from concourse.bass_utils import run_bass_kernel_spmd

    y = np.ascontiguousarray(np.asarray(y, dtype=np.float32))
    b = np.ascontiguousarray(np.asarray(b, dtype=np.float32))
    A = np.asarray(A, dtype=np.float32)
    assert y.shape == (BATCH, IN_DIM) and b.shape == (BATCH, OUT_DIM)

    Wc = _build_weights(A)
    yh = y.astype(np.float16)
    bh = b.astype(np.float16)

    in_maps = []
    for core in range(N_CORES):
        sl = slice(core * SHARD, (core + 1) * SHARD)
        in_maps.append({"In": _pack_in(yh[sl], bh[sl]), "Wc": Wc})

    nc = _get_program()
    res = run_bass_kernel_spmd(nc, in_maps, core_ids=list(range(N_CORES)))
    last_results = res

    out = np.empty((BATCH, IN_DIM), np.float32)
    for core in range(N_CORES):
        out[core * SHARD:(core + 1) * SHARD] = _unpack_out(res.results[core]["Ot"])
    return out
